# revision 41
# baseline (speedup 1.0000x reference)
"""Memory-bank attention read on 8 NeuronCores (Trainium2, Bass/Tile).

out[b] = softmax(q_b @ K^T, axis=m) @ K  per batch b, sharded batch->core.

Layout trick: query/output are NCHW, so query[b].reshape(256, 4096) is already
q^T in [d, n] form -- exactly the moving-operand layout the TensorEngine wants.
The whole kernel runs in "transposed" space (scoreT [m, n], outT [d, n]):
  mm1:  scoreT[mi] = kT_chunk.T @ qT_chunk   fp16 operands (1 cyc/row, halves
        the input DMA + H2D bytes vs fp32; score accumulates exactly in PSUM)
  exp:  expT = exp(scoreT - 40) on ScalarE -> bf16 (global shift; bf16 keeps
        fp32's exponent range so e^{score-40} up to ~e^50 cannot overflow)
  mm2:  outT += keys_chunk.T @ expT           bf16 operands
  rs:   GpSimd (Pool engine, otherwise idle) accumulates the 16 exp tiles of
        a chunk into acc[128, n] in fp32; ONE ones^T @ acc matmul per chunk
        then yields the softmax denominator broadcast to every partition --
        replacing 16 accumulating ones-matmuls (20% of all PE cycles).
  out:  outT * (1/rowsum) on VectorE -> fp16, one batched DMA per chunk.
"""

import numpy as np
import ml_dtypes

import concourse.bass as bass
import concourse.bacc as bacc
import concourse.mybir as mybir
import concourse.tile as tile
from concourse import bass_utils

B, D, HH, WW = 8, 256, 64, 64
N = HH * WW            # 4096 queries per core
M = 2048               # memory slots
NCH = 512              # n-chunk (1 PSUM bank at fp32)
NCHUNKS = N // NCH     # 8
MT = M // 128          # 16 m-tiles
SHIFT = -40.0          # global softmax shift

F32 = mybir.dt.float32
F32R = mybir.dt.float32r
F16 = mybir.dt.float16
BF16 = mybir.dt.bfloat16

_cached_nc = {}


def _build(repeat=1):
    key = repeat
    if key in _cached_nc:
        return _cached_nc[key]

    nc = bacc.Bacc("TRN2", target_bir_lowering=False, debug=False, num_devices=B)
    qT_d = nc.dram_tensor("qT", [D, N], F16, kind="ExternalInput").ap()
    kT_d = nc.dram_tensor("keysT", [D, M], F16, kind="ExternalInput").ap()
    k_d = nc.dram_tensor("keys", [M, D], BF16, kind="ExternalInput").ap()
    o_d = nc.dram_tensor("outT", [D, N], F16, kind="ExternalOutput").ap()

    with tile.TileContext(nc) as tc:
        with (
            tc.tile_pool(name="big", bufs=1) as big,
            tc.tile_pool(name="qp", bufs=2) as qp,
            tc.tile_pool(name="expp", bufs=6) as expp,
            tc.tile_pool(name="accp", bufs=2) as accp,
            tc.tile_pool(name="evp", bufs=2) as evp,
            tc.tile_pool(name="ps_s", bufs=3, space=bass.MemorySpace.PSUM) as ps_s,
            tc.tile_pool(name="ps_o", bufs=2, space=bass.MemorySpace.PSUM) as ps_o,
            tc.tile_pool(name="ps_r", bufs=1, space=bass.MemorySpace.PSUM) as ps_r,
        ):
            kT = big.tile([128, 2, M], F16)    # [:, h, :] = keysT rows h*128..
            ks = big.tile([128, MT, D], BF16)  # [:, t, :] = keys rows t*128..
            ones_bf = big.tile([128, 128], BF16)
            bias = big.tile([128, 1], F32)

            # kT's first m-tiles + the first q chunk are the critical path;
            # the rest of kT and ks (mm2 operand) stream in behind them.
            kT_r = kT_d.rearrange("(h p) m -> p h m", p=128)
            nc.sync.dma_start(kT[:, 0, 0:512], kT_r[:, 0, 0:512])
            nc.vector.memset(ones_bf[:], 1.0)
            nc.vector.memset(bias[:], SHIFT)

            for rep in range(repeat):
                pending_rs = [None]  # deferred rowsum matmul of previous chunk

                for nch in range(NCHUNKS):
                    nsl = slice(nch * NCH, (nch + 1) * NCH)
                    qTc = qp.tile([128, 2, NCH], F16, tag="qTc")
                    q_r = qT_d[:, nsl].rearrange("(h p) n -> p h n", p=128)
                    if rep == 0 and nch == 0:
                        # critical-path h=0 pieces first so mm1(0) h0 can
                        # start after ~256KB instead of the full 1MB
                        nc.sync.dma_start(qTc[:, 0, :], q_r[:, 0, :])
                        nc.sync.dma_start(qTc[:, 1, :], q_r[:, 1, :])
                        nc.sync.dma_start(kT[:, 1, 0:512], kT_r[:, 1, 0:512])
                        nc.sync.dma_start(kT[:, :, 512:M], kT_r[:, :, 512:M])
                        nc.sync.dma_start(
                            ks[:], k_d.rearrange("(t p) d -> p t d", p=128)
                        )
                    else:
                        nc.sync.dma_start(qTc[:], q_r)
                    out0 = ps_o.tile([128, NCH], F32, tag="out0")
                    out1 = ps_o.tile([128, NCH], F32, tag="out1")
                    # exp-tile accumulators, split across the two otherwise
                    # idle SIMD engines so neither chain throttles ScalarE
                    acc_p = accp.tile([128, NCH], F32, tag="acc_p")
                    acc_v = accp.tile([128, NCH], F32, tag="acc_v")
                    expts = [None] * MT
                    scores = [None] * MT

                    def mm1(i):
                        sc = ps_s.tile([128, NCH], F32, tag="score")
                        for h in range(2):
                            nc.tensor.matmul(
                                sc[:],
                                kT[:, h, i * 128:(i + 1) * 128],
                                qTc[:, h, :],
                                start=(h == 0),
                                stop=(h == 1),
                            )
                        scores[i] = sc

                    last = nch == NCHUNKS - 1

                    def do_exp(i):
                        e = expp.tile([128, NCH], BF16, tag="expt")
                        nc.scalar.activation(
                            e[:], scores[i][:], mybir.ActivationFunctionType.Exp,
                            bias=bias[:], scale=1.0,
                        )
                        expts[i] = e
                        if last and i >= MT - 2:
                            # final chunk: last two tiles join the rowsum via
                            # direct PE matmuls, off the accumulator chain
                            return
                        eng = nc.gpsimd if i % 2 == 0 else nc.vector
                        a = acc_p if i % 2 == 0 else acc_v
                        if i < 2:
                            eng.tensor_copy(a[:], e[:])
                        else:
                            eng.tensor_add(a[:], a[:], e[:])

                    def mm2(i):
                        e = expts[i][:]
                        st, sp = (i == 0), (i == MT - 1)
                        nc.tensor.matmul(out0[:], ks[:, i, 0:128], e,
                                         start=st, stop=sp)
                        nc.tensor.matmul(out1[:], ks[:, i, 128:256], e,
                                         start=st, stop=sp)

                    accb_l = [None]
                    for i in range(MT):
                        mm1(i)
                        # rowsum matmul of the PREVIOUS chunk: emitted early in
                        # this chunk's PE stream so the engine never stalls on
                        # the Pool accumulator finishing at a chunk boundary.
                        if i == 6 and pending_rs[0] is not None:
                            pending_rs[0]()
                            pending_rs[0] = None
                        if last and i == MT - 1:
                            # merge tiles 0..13 while 14/15 are still in flight
                            accb = evp.tile([128, NCH], BF16, tag="accb")
                            nc.vector.tensor_add(accb[:], acc_p[:], acc_v[:])
                            accb_l[0] = accb
                        do_exp(i)
                        if i >= 2:
                            mm2(i - 2)
                    if not last:
                        mm2(MT - 2)
                        mm2(MT - 1)

                    def finish(acc_p=acc_p, acc_v=acc_v, out0=out0, out1=out1,
                               nsl=nsl):
                        # merge rounds to bf16 on write (adds computed in
                        # fp32) so the rowsum matmul runs at 1 cyc/row
                        accb = evp.tile([128, NCH], BF16, tag="accb")
                        nc.vector.tensor_add(accb[:], acc_p[:], acc_v[:])
                        rs = ps_r.tile([128, NCH], F32, tag="rs")
                        nc.tensor.matmul(
                            rs[:], ones_bf[:], accb[:],
                            start=True, stop=True,
                        )
                        recip = evp.tile([128, NCH], F32, tag="recip")
                        o01 = evp.tile([128, 2, NCH], F16, tag="o01")
                        nc.vector.reciprocal_approx_fast(recip[:], rs[:])
                        nc.vector.tensor_mul(o01[:, 0, :], out0[:], recip[:])
                        nc.vector.tensor_mul(o01[:, 1, :], out1[:], recip[:])
                        nc.sync.dma_start(
                            o_d[:, nsl].rearrange("(h p) n -> p h n", p=128),
                            o01[:],
                        )

                    if not last:
                        pending_rs[0] = finish
                        continue

                    # final chunk: rowsum = ones @ (accb + e14 + e15), with
                    # the partial landing on PE between the trailing mm2s so
                    # nothing waits at the drain.
                    rs = ps_r.tile([128, NCH], F32, tag="rs")
                    nc.tensor.matmul(rs[:], ones_bf[:], accb_l[0][:],
                                     start=True, stop=False)
                    mm2(MT - 2)
                    nc.tensor.matmul(rs[:], ones_bf[:], expts[MT - 2][:],
                                     start=False, stop=False)
                    mm2(MT - 1)
                    nc.tensor.matmul(rs[:], ones_bf[:], expts[MT - 1][:],
                                     start=False, stop=True)
                    recip = evp.tile([128, NCH], F32, tag="recip")
                    o01 = evp.tile([128, 2, NCH], F16, tag="o01")
                    nc.vector.reciprocal_approx_fast(recip[:], rs[:])
                    nc.vector.tensor_mul(o01[:, 0, :], out0[:], recip[:])
                    nc.vector.tensor_mul(o01[:, 1, :], out1[:], recip[:])
                    nc.sync.dma_start(
                        o_d[:, nsl].rearrange("(h p) n -> p h n", p=128),
                        o01[:],
                    )

    nc.compile()
    _cached_nc[key] = nc
    return nc


def _in_maps(keys, query):
    keys = np.asarray(keys, dtype=np.float32)
    q = np.asarray(query, dtype=np.float32)
    kT16 = np.ascontiguousarray(keys.T).astype(np.float16)
    kb16 = keys.astype(ml_dtypes.bfloat16)
    q16 = q.reshape(B, D, N).astype(np.float16)
    return [
        {"qT": q16[b], "keysT": kT16, "keys": kb16}
        for b in range(B)
    ]


def _run(keys, query, trace=False, repeat=1, **trace_kwargs):
    nc = _build(repeat)
    return bass_utils.run_bass_kernel_spmd(
        nc, _in_maps(keys, query), core_ids=list(range(B)), trace=trace,
        **trace_kwargs
    )


def kernel(keys, query, value):
    res = _run(keys, query)
    out = np.stack([res.results[b]["outT"] for b in range(B)])  # [B, D, N] f16
    return np.ascontiguousarray(
        out.astype(np.float32).reshape(B, D, HH, WW)
    )


# revision 46
# speedup vs baseline: 1.0007x; 1.0007x over previous
"""Memory-bank attention read on 8 NeuronCores (Trainium2, Bass/Tile).

out[b] = softmax(q_b @ K^T, axis=m) @ K  per batch b, sharded batch->core.

Layout trick: query/output are NCHW, so query[b].reshape(256, 4096) is already
q^T in [d, n] form -- exactly the moving-operand layout the TensorEngine wants.
The whole kernel runs in "transposed" space (scoreT [m, n], outT [d, n]):
  mm1:  scoreT[mi] = kT_chunk.T @ qT_chunk   fp16 operands (1 cyc/row, halves
        the input DMA + H2D bytes vs fp32; score accumulates exactly in PSUM)
  exp:  expT = exp(scoreT - 40) on ScalarE -> bf16 (global shift; bf16 keeps
        fp32's exponent range so e^{score-40} up to ~e^50 cannot overflow)
  mm2:  outT += keys_chunk.T @ expT           bf16 operands
  rs:   GpSimd (Pool engine, otherwise idle) accumulates the 16 exp tiles of
        a chunk into acc[128, n] in fp32; ONE ones^T @ acc matmul per chunk
        then yields the softmax denominator broadcast to every partition --
        replacing 16 accumulating ones-matmuls (20% of all PE cycles).
  out:  outT * (1/rowsum) on VectorE -> fp16, one batched DMA per chunk.
"""

import numpy as np
import ml_dtypes

import concourse.bass as bass
import concourse.bacc as bacc
import concourse.mybir as mybir
import concourse.tile as tile
from concourse import bass_utils

B, D, HH, WW = 8, 256, 64, 64
N = HH * WW            # 4096 queries per core
M = 2048               # memory slots
NCH = 512              # n-chunk (1 PSUM bank at fp32)
NCHUNKS = N // NCH     # 8
MT = M // 128          # 16 m-tiles
SHIFT = -40.0          # global softmax shift

F32 = mybir.dt.float32
F32R = mybir.dt.float32r
F16 = mybir.dt.float16
BF16 = mybir.dt.bfloat16

_cached_nc = {}


def _build(repeat=1):
    key = repeat
    if key in _cached_nc:
        return _cached_nc[key]

    nc = bacc.Bacc("TRN2", target_bir_lowering=False, debug=False, num_devices=B)
    qT_d = nc.dram_tensor("qT", [D, N], F16, kind="ExternalInput").ap()
    kT_d = nc.dram_tensor("keysT", [D, M], F16, kind="ExternalInput").ap()
    k_d = nc.dram_tensor("keys", [M, D], BF16, kind="ExternalInput").ap()
    o_d = nc.dram_tensor("outT", [D, N], F16, kind="ExternalOutput").ap()

    with tile.TileContext(nc) as tc:
        with (
            tc.tile_pool(name="big", bufs=1) as big,
            tc.tile_pool(name="qp", bufs=2) as qp,
            tc.tile_pool(name="expp", bufs=6) as expp,
            tc.tile_pool(name="accp", bufs=2) as accp,
            tc.tile_pool(name="evp", bufs=2) as evp,
            tc.tile_pool(name="ps_s", bufs=3, space=bass.MemorySpace.PSUM) as ps_s,
            tc.tile_pool(name="ps_o", bufs=2, space=bass.MemorySpace.PSUM) as ps_o,
            tc.tile_pool(name="ps_r", bufs=1, space=bass.MemorySpace.PSUM) as ps_r,
        ):
            kT = big.tile([128, 2, M], F16)    # [:, h, :] = keysT rows h*128..
            ks = big.tile([128, MT, D], BF16)  # [:, t, :] = keys rows t*128..
            ones_bf = big.tile([128, 128], BF16)
            bias = big.tile([128, 1], F32)

            # kT's first m-tiles + the first q chunk are the critical path;
            # the rest of kT and ks (mm2 operand) stream in behind them.
            kT_r = kT_d.rearrange("(h p) m -> p h m", p=128)
            nc.sync.dma_start(kT[:, 0, 0:512], kT_r[:, 0, 0:512])
            nc.vector.memset(ones_bf[:], 1.0)
            nc.vector.memset(bias[:], SHIFT)

            for rep in range(repeat):
                pending_rs = [None]  # deferred rowsum matmul of previous chunk
                pending_mm2 = []     # previous chunk's trailing mm2 closures

                for nch in range(NCHUNKS):
                    nsl = slice(nch * NCH, (nch + 1) * NCH)
                    qTc = qp.tile([128, 2, NCH], F16, tag="qTc")
                    q_r = qT_d[:, nsl].rearrange("(h p) n -> p h n", p=128)
                    if rep == 0 and nch == 0:
                        # critical-path h=0 pieces first so mm1(0) h0 can
                        # start after ~256KB instead of the full 1MB
                        nc.sync.dma_start(qTc[:, 0, :], q_r[:, 0, :])
                        nc.sync.dma_start(qTc[:, 1, :], q_r[:, 1, :])
                        nc.sync.dma_start(kT[:, 1, 0:512], kT_r[:, 1, 0:512])
                        nc.sync.dma_start(kT[:, :, 512:M], kT_r[:, :, 512:M])
                        nc.sync.dma_start(
                            ks[:], k_d.rearrange("(t p) d -> p t d", p=128)
                        )
                    else:
                        nc.sync.dma_start(qTc[:], q_r)
                    out0 = ps_o.tile([128, NCH], F32, tag="out0")
                    out1 = ps_o.tile([128, NCH], F32, tag="out1")
                    # exp-tile accumulators, split across the two otherwise
                    # idle SIMD engines so neither chain throttles ScalarE
                    acc_p = accp.tile([128, NCH], F32, tag="acc_p")
                    acc_v = accp.tile([128, NCH], F32, tag="acc_v")
                    expts = [None] * MT
                    scores = [None] * MT

                    def mm1(i):
                        sc = ps_s.tile([128, NCH], F32, tag="score")
                        for h in range(2):
                            nc.tensor.matmul(
                                sc[:],
                                kT[:, h, i * 128:(i + 1) * 128],
                                qTc[:, h, :],
                                start=(h == 0),
                                stop=(h == 1),
                            )
                        scores[i] = sc

                    last = nch == NCHUNKS - 1

                    def do_exp(i):
                        e = expp.tile([128, NCH], BF16, tag="expt")
                        nc.scalar.activation(
                            e[:], scores[i][:], mybir.ActivationFunctionType.Exp,
                            bias=bias[:], scale=1.0,
                        )
                        expts[i] = e
                        if last and i >= MT - 4:
                            # final chunk: last four tiles join the rowsum via
                            # direct PE matmuls, off the accumulator chain, so
                            # the drain never waits on the SIMD engines
                            return
                        eng = nc.gpsimd if i % 2 == 0 else nc.vector
                        a = acc_p if i % 2 == 0 else acc_v
                        if i < 2:
                            eng.tensor_copy(a[:], e[:])
                        else:
                            eng.tensor_add(a[:], a[:], e[:])

                    def mm2(i, out0=out0, out1=out1, expts=expts):
                        e = expts[i][:]
                        st, sp = (i == 0), (i == MT - 1)
                        nc.tensor.matmul(out0[:], ks[:, i, 0:128], e,
                                         start=st, stop=sp)
                        nc.tensor.matmul(out1[:], ks[:, i, 128:256], e,
                                         start=st, stop=sp)

                    accb_l = [None]
                    for i in range(MT):
                        mm1(i)
                        # rowsum matmul of the PREVIOUS chunk: emitted early in
                        # this chunk's PE stream so the engine never stalls on
                        # the Pool accumulator finishing at a chunk boundary.
                        if i == 6 and pending_rs[0] is not None:
                            pending_rs[0]()
                            pending_rs[0] = None
                        if last and i == MT - 2:
                            # merge tiles 0..11 while 12..15 are in flight
                            accb = evp.tile([128, NCH], BF16, tag="accb")
                            nc.vector.tensor_add(accb[:], acc_p[:], acc_v[:])
                            accb_l[0] = accb
                        do_exp(i)
                        # mm2 runs two tiles behind mm1, rolling ACROSS chunk
                        # boundaries so the pipeline never refills from empty
                        if i >= 2:
                            mm2(i - 2)
                        elif pending_mm2:
                            pending_mm2.pop(0)()

                    def finish(acc_p=acc_p, acc_v=acc_v, out0=out0, out1=out1,
                               nsl=nsl):
                        # merge rounds to bf16 on write (adds computed in
                        # fp32) so the rowsum matmul runs at 1 cyc/row
                        accb = evp.tile([128, NCH], BF16, tag="accb")
                        nc.vector.tensor_add(accb[:], acc_p[:], acc_v[:])
                        rs = ps_r.tile([128, NCH], F32, tag="rs")
                        nc.tensor.matmul(
                            rs[:], ones_bf[:], accb[:],
                            start=True, stop=True,
                        )
                        recip = evp.tile([128, NCH], F32, tag="recip")
                        o01 = evp.tile([128, 2, NCH], F16, tag="o01")
                        nc.vector.reciprocal_approx_fast(recip[:], rs[:])
                        nc.vector.tensor_mul(o01[:, 0, :], out0[:], recip[:])
                        nc.vector.tensor_mul(o01[:, 1, :], out1[:], recip[:])
                        nc.sync.dma_start(
                            o_d[:, nsl].rearrange("(h p) n -> p h n", p=128),
                            o01[:],
                        )

                    if not last:
                        pending_mm2[:] = [
                            lambda f=mm2: f(MT - 2),
                            lambda f=mm2: f(MT - 1),
                        ]
                        pending_rs[0] = finish
                        continue

                    # final chunk drain: rowsum = ones @ (accb + e12..e15);
                    # the partials land on PE between the trailing mm2s so
                    # nothing waits on the SIMD engines.
                    rs = ps_r.tile([128, NCH], F32, tag="rs")
                    nc.tensor.matmul(rs[:], ones_bf[:], accb_l[0][:],
                                     start=True, stop=False)
                    mm2(MT - 2)
                    nc.tensor.matmul(rs[:], ones_bf[:], expts[MT - 4][:],
                                     start=False, stop=False)
                    nc.tensor.matmul(rs[:], ones_bf[:], expts[MT - 3][:],
                                     start=False, stop=False)
                    mm2(MT - 1)
                    nc.tensor.matmul(rs[:], ones_bf[:], expts[MT - 2][:],
                                     start=False, stop=False)
                    nc.tensor.matmul(rs[:], ones_bf[:], expts[MT - 1][:],
                                     start=False, stop=True)
                    recip = evp.tile([128, NCH], F32, tag="recip")
                    o01 = evp.tile([128, 2, NCH], F16, tag="o01")
                    nc.vector.reciprocal_approx_fast(recip[:], rs[:])
                    nc.vector.tensor_mul(o01[:, 0, :], out0[:], recip[:])
                    nc.vector.tensor_mul(o01[:, 1, :], out1[:], recip[:])
                    nc.sync.dma_start(
                        o_d[:, nsl].rearrange("(h p) n -> p h n", p=128),
                        o01[:],
                    )

    nc.compile()
    _cached_nc[key] = nc
    return nc


def _in_maps(keys, query):
    keys = np.asarray(keys, dtype=np.float32)
    q = np.asarray(query, dtype=np.float32)
    kT16 = np.ascontiguousarray(keys.T).astype(np.float16)
    kb16 = keys.astype(ml_dtypes.bfloat16)
    q16 = q.reshape(B, D, N).astype(np.float16)
    return [
        {"qT": q16[b], "keysT": kT16, "keys": kb16}
        for b in range(B)
    ]


def _run(keys, query, trace=False, repeat=1, **trace_kwargs):
    nc = _build(repeat)
    return bass_utils.run_bass_kernel_spmd(
        nc, _in_maps(keys, query), core_ids=list(range(B)), trace=trace,
        **trace_kwargs
    )


def kernel(keys, query, value):
    res = _run(keys, query)
    out = np.stack([res.results[b]["outT"] for b in range(B)])  # [B, D, N] f16
    return np.ascontiguousarray(
        out.astype(np.float32).reshape(B, D, HH, WW)
    )
